# revision 23
# baseline (speedup 1.0000x reference)
"""Trainium2 Bass kernel for nn_BrickVectorEdgeModel (GNN edge MLP).

Computes, for each batch b and node pair (i, j):
    f   = relu(W_b @ relu(W_a @ bv + b_a + W_xy @ xy + b_xy) + b_b)   per node
    e1  = relu(W1 @ f[j] + W2 @ f[i] + b_ca)                          per edge
    e2  = relu(W_cb @ e1 + b_cb)
    e3  = relu(W_cc @ e2 + b_cc)
    out = W_out @ e3 + b_out                                          (2 channels)

Sharding: the (B=4, N=192) x N edge grid has 768 i-rows; each of the 8
cores takes 96 consecutive rows, which always fall inside a single batch
b = core//2.  Host permutes that batch's 192 nodes so the core's own 96
i-rows come first; every core then runs the identical program (SPMD) on
its own node set.  Matmuls run in fp16 (fp32 PSUM accumulate).

The kernel is PE-streaming-bound (~99.6% tensor occupancy), so the focus
is trimming non-stream time:
 - node phase streams 192 cols (true node count) instead of 256-pad;
   vpb streams only the core's own 96 rows.
 - xy contribution enters as a 2-partition matmul (no 128-row zero pad),
   W_out as a 2-column stationary (psum rows 0-1 only).
 - weight DMA is split across three queues (scalar-HWDGE carries the
   f1 critical path, gpsimd-SWDGE the f2/u/v weights, sync the edge
   weights) in consumption order, so the first matmul starts as soon as
   the first ~180KB land instead of after the full 3.4MB.
 - a few zero warmup matmuls run during the initial DMA wait to start
   the tensor-engine DVFS ramp early.
"""

import numpy as np

import concourse.bass as bass
import concourse.mybir as mybir
import concourse.tile as tile
from concourse import bacc
from concourse.bass_utils import run_bass_kernel_spmd

P = 128
H = 512          # hidden width
D = 512          # brick vector dim
B = 4
N = 192          # nodes per batch
NCORES = 8
RLOC = 96        # edge-grid rows per core
EDGES = RLOC * N             # flat edge columns per core (18432)
CHUNK = 512
NCHUNK = EDGES // CHUNK      # 36
VW = RLOC        # vpb columns (own rows only)

WCOLS = 24 * H + 4 * P       # wcat4|wb|w1|w2|wcb|wcc (4*H each) + wout (4*128)

# bias blob layout (fp32): [128 x BCOLS]
_blayout = [("b1", 4), ("bb", 4), ("bca", 4), ("bcb", 4), ("bcc", 4), ("bout", 1)]
BOFF = {}
_c = 0
for _n, _s in _blayout:
    BOFF[_n] = (_c, _s)
    _c += _s
BCOLS = _c


def _to_tiles(w):
    """[K, M] (K = 4*128 contraction) -> [128, 4, M] stationary layout."""
    K, M = w.shape
    return w.reshape(K // P, P, M).transpose(1, 0, 2)


def _pack_weights(W_xy, b_xy, W_a, b_a, W_b, b_b, W_ca, b_ca, W_cb, b_cb,
                  W_cc, b_cc, W_out, b_out):
    blob = np.zeros((P, WCOLS), np.float16)

    def put(idx, arr3):  # arr3: [128, nk, M] -> block idx (4*H cols each)
        blob[:, idx * 4 * H:(idx + 1) * 4 * H] = \
            arr3.reshape(P, -1).astype(np.float16)

    put(0, _to_tiles(W_a.T.astype(np.float32)))       # wcat4 (bv part only)
    put(1, _to_tiles(W_b.T.astype(np.float32)))
    W1, W2 = W_ca[:, :H], W_ca[:, H:]
    put(2, _to_tiles(W1.T.astype(np.float32)))
    put(3, _to_tiles(W2.T.astype(np.float32)))
    put(4, _to_tiles(W_cb.T.astype(np.float32)))
    put(5, _to_tiles(W_cc.T.astype(np.float32)))
    wout = np.zeros((H, P), np.float32)
    wout[:, 0:2] = W_out.T.astype(np.float32)
    blob[:, 24 * H:] = _to_tiles(wout).reshape(P, -1).astype(np.float16)

    bblob = np.zeros((P, BCOLS), np.float32)

    def putb(name, vec):  # [512] -> [128, 4]
        off, sz = BOFF[name]
        bblob[:, off:off + sz] = vec.astype(np.float32).reshape(4, P).T

    putb("b1", np.asarray(b_a) + np.asarray(b_xy))
    putb("bb", b_b)
    putb("bca", b_ca)
    putb("bcb", b_cb)
    putb("bcc", b_cc)
    off, _ = BOFF["bout"]
    bblob[0:2, off] = np.asarray(b_out, np.float32)
    return blob, bblob


def _pack_nodes(W_a, bv_b, perm):
    """Per-core stage1 blob [128, 4, H+N] fp16: [wcat_k | bv^T_k] per k."""
    s = np.zeros((P, 4, H + N), np.float16)
    s[:, :, :H] = _to_tiles(W_a.T.astype(np.float32)).astype(np.float16)
    bvT = bv_b[perm].T.astype(np.float32)          # [512, 192]
    s[:, :, H:] = bvT.reshape(4, P, N).transpose(1, 0, 2).astype(np.float16)
    return s


def _pack_xy(W_xy, xy_b, perm):
    """Per-core 2-partition strip [2, H + N]: W_xy^T | xy^T."""
    s = np.zeros((2, H + N), np.float16)
    s[:, :H] = W_xy.T.astype(np.float16)
    s[:, H:] = xy_b[perm].T.astype(np.float16)
    return s


def make_in_maps(brick_vectors, xy, W_xy, b_xy, W_a, b_a, W_b, b_b,
                 W_ca, b_ca, W_cb, b_cb, W_cc, b_cc, W_out, b_out):
    blob, bblob = _pack_weights(W_xy, b_xy, W_a, b_a, W_b, b_b, W_ca, b_ca,
                                W_cb, b_cb, W_cc, b_cc, W_out, b_out)
    perms, in_maps = [], []
    for c in range(NCORES):
        b, half = c // 2, c % 2
        perm = np.concatenate([np.arange(96) + 96 * half,
                               np.arange(96) + 96 * (1 - half)])
        perms.append((b, perm))
        in_maps.append({
            "wblob": blob,
            "bblob": bblob,
            "nodes": _pack_nodes(np.asarray(W_a),
                                 np.asarray(brick_vectors, np.float32)[b],
                                 perm),
            "xyblob": _pack_xy(np.asarray(W_xy), np.asarray(xy, np.float32)[b],
                               perm),
        })
    return perms, in_maps


def _build():
    f32 = mybir.dt.float32
    Relu = mybir.ActivationFunctionType.Relu
    add = mybir.AluOpType.add
    amax = mybir.AluOpType.max

    f16 = mybir.dt.float16
    nc = bacc.Bacc(None, target_bir_lowering=False)
    wblob = nc.declare_dram_parameter("wblob", [P, WCOLS], f16, isOutput=False)
    bblob = nc.declare_dram_parameter("bblob", [P, BCOLS], f32, isOutput=False)
    nodes = nc.declare_dram_parameter("nodes", [P, 4, H + N], f16,
                                      isOutput=False)
    xyblob = nc.declare_dram_parameter("xyblob", [2, H + N], f16, isOutput=False)
    y = nc.declare_dram_parameter("y", [2, EDGES], f32, isOutput=True)

    with tile.TileContext(nc) as tc:
        with tc.tile_pool(name="wf", bufs=1) as wf, \
             tc.tile_pool(name="stp", bufs=1) as stp, \
             tc.tile_pool(name="wr", bufs=1) as wr, \
             tc.tile_pool(name="ep", bufs=2) as ep, \
             tc.tile_pool(name="outp", bufs=3) as outp, \
             tc.tile_pool(name="psA", bufs=4, space="PSUM") as psA, \
             tc.tile_pool(name="psB", bufs=4, space="PSUM") as psB:

            # warmup operands (zeroed so CoreSim sees defined data)
            warm_s = wf.tile([P, 32], f16, tag="warm_s")
            warm_m = wf.tile([P, CHUNK], f16, tag="warm_m")
            nc.gpsimd.memset(warm_s[:], 0.0)
            nc.gpsimd.memset(warm_m[:], 0.0)

            # stage1 holds [wcat_k | nodes_k] per k-tile: one 5632B-row DMA
            st1 = stp.tile([P, 4, H + N], f16, tag="st1")
            xyt = wf.tile([2, H + N], f16, tag="xyt")
            biast = wf.tile([P, BCOLS], f32, tag="bias")
            wbt = stp.tile([P, 4, H], f16, tag="wb")
            w1t = stp.tile([P, 4, H], f16, tag="w1")
            w2t = stp.tile([P, 4, H], f16, tag="w2")
            wcbt = stp.tile([P, 4, H], f16, tag="wcb")
            wcct = stp.tile([P, 4, H], f16, tag="wcc")
            woutt = stp.tile([P, 4, P], f16, tag="wout")

            # ---- DMA triggers: sync HWDGE (fast, ~190GB/s w/ 4KB rows)
            # carries the critical path in consumption order; scalar HWDGE
            # and gpsimd SWDGE prefetch later-consumed weights in parallel.
            nc.sync.dma_start(st1[:], nodes[:])
            nc.sync.dma_start(wbt[:], wblob[:, 4 * H:8 * H])
            nc.sync.dma_start(wcbt[:], wblob[:, 16 * H:20 * H])
            nc.scalar.dma_start(biast[:], bblob[:])
            nc.scalar.dma_start(xyt[:], xyblob[:])
            nc.scalar.dma_start(w2t[:], wblob[:, 12 * H:16 * H])
            nc.scalar.dma_start(woutt[:], wblob[:, 24 * H:])
            nc.gpsimd.dma_start(w1t[:], wblob[:, 8 * H:12 * H])
            nc.gpsimd.dma_start(wcct[:], wblob[:, 20 * H:24 * H])

            def bias(name, m):
                off, _ = BOFF[name]
                return biast[:, off + m:off + m + 1]

            # ---- PE warmup: start the DVFS ramp during the DMA wait ----
            pw = psA.tile([P, CHUNK], f32, tag="psA")
            for _ in range(8):
                nc.tensor.matmul(pw[:32, :CHUNK], warm_s[:, :32], warm_m[:],
                                 start=True, stop=True)

            # ---- node phase ----
            # f1 (k-outer so matmuls gate on per-k weight slices)
            f1 = wr.tile([P, 4, N], f16, tag="f1")
            pts1 = [psA.tile([P, CHUNK], f32, tag="psA", name=f"pts1_{m}")
                    for m in range(4)]
            for k in range(4):
                for m in range(4):
                    nc.tensor.matmul(pts1[m][:, :N],
                                     st1[:, k, m * P:(m + 1) * P],
                                     st1[:, k, H:], start=(k == 0), stop=False)
            for m in range(4):
                nc.tensor.matmul(pts1[m][:, :N], xyt[0:2, m * P:(m + 1) * P],
                                 xyt[0:2, H:H + N], start=False, stop=True)
            for m in range(4):
                if m % 2 == 0:
                    nc.scalar.activation(f1[:, m, :], pts1[m][:, :N], Relu,
                                         bias=bias("b1", m), scale=1.0)
                else:
                    nc.vector.tensor_scalar(f1[:, m, :], pts1[m][:, :N],
                                            bias("b1", m), 0.0, add, amax)

            # keep the PE busy through the wbt DMA wait (DVFS ramp)
            pw2 = psB.tile([P, CHUNK], f32, tag="psB")
            for _ in range(3):
                nc.tensor.matmul(pw2[:32, :CHUNK], warm_s[:, :32], warm_m[:],
                                 start=True, stop=True)

            # f2
            f2 = wr.tile([P, 4, N], f16, tag="f2")
            pts2 = [psB.tile([P, CHUNK], f32, tag="psB", name=f"pts2_{m}")
                    for m in range(4)]
            for k in range(4):
                for m in range(4):
                    nc.tensor.matmul(pts2[m][:, :N], wbt[:, k, m * P:(m + 1) * P],
                                     f1[:, k, :], start=(k == 0), stop=(k == 3))
            for m in range(4):
                if m % 2 == 0:
                    nc.scalar.activation(f2[:, m, :], pts2[m][:, :N], Relu,
                                         bias=bias("bb", m), scale=1.0)
                else:
                    nc.vector.tensor_scalar(f2[:, m, :], pts2[m][:, :N],
                                            bias("bb", m), 0.0, add, amax)

            # u (all 192 j-nodes) and vpb = W2@f2 + b_ca (own 96 i-rows only)
            u = wr.tile([P, 4, N], f32, tag="u")
            vpb = wr.tile([P, 4, VW], f32, tag="vpb")
            for mm in range(4):
                pu = psA.tile([P, CHUNK], f32, tag="psA")
                for k in range(4):
                    nc.tensor.matmul(pu[:, :N], w1t[:, k, mm * P:(mm + 1) * P],
                                     f2[:, k, :], start=(k == 0), stop=(k == 3))
                if mm < 2:
                    nc.scalar.copy(u[:, mm, :], pu[:, :N])
                else:
                    nc.vector.tensor_copy(u[:, mm, :], pu[:, :N])
                pv = psB.tile([P, CHUNK], f32, tag="psB")
                for k in range(4):
                    nc.tensor.matmul(pv[:, :VW],
                                     w2t[:, k, mm * P:(mm + 1) * P],
                                     f2[:, k, :VW], start=(k == 0), stop=(k == 3))
                nc.vector.tensor_scalar_add(vpb[:, mm, :], pv[:, :VW],
                                            bias("bca", mm))

            # ---- edge phase: 512-wide chunks; last chunk split in two to
            #      shorten the serial relu->out tail after the final matmul ----
            chunk_list = [(cc * CHUNK, CHUNK) for cc in range(NCHUNK - 1)]
            chunk_list += [((NCHUNK - 1) * CHUNK, CHUNK // 2),
                           ((NCHUNK - 1) * CHUNK + CHUNK // 2, CHUNK // 2)]
            for cc, (f0, cw) in enumerate(chunk_list):
                e1 = ep.tile([P, 4, CHUNK], f16, tag="e1")
                r_lo = f0 // N
                r_hi = (f0 + cw - 1) // N
                for kt in range(4):
                    for rl in range(r_lo, r_hi + 1):
                        cs = max(f0, rl * N)
                        ce = min(f0 + cw, (rl + 1) * N)
                        if cc == 0 and kt >= 2:
                            nc.vector.tensor_scalar(
                                e1[:, kt, cs - f0:ce - f0],
                                u[:, kt, cs - rl * N:ce - rl * N],
                                vpb[:, kt, rl:rl + 1], 0.0, add, amax)
                        else:
                            nc.scalar.activation(
                                e1[:, kt, cs - f0:ce - f0],
                                u[:, kt, cs - rl * N:ce - rl * N],
                                Relu, bias=vpb[:, kt, rl:rl + 1], scale=1.0)

                e2 = ep.tile([P, 4, CHUNK], f16, tag="e2")
                for m in range(4):
                    pt = psA.tile([P, CHUNK], f32, tag="psA")
                    for k in range(4):
                        nc.tensor.matmul(pt[:, :cw], wcbt[:, k, m * P:(m + 1) * P],
                                         e1[:, k, :cw], start=(k == 0), stop=(k == 3))
                    nc.vector.tensor_scalar(e2[:, m, :cw], pt[:, :cw],
                                            bias("bcb", m), 0.0, add, amax)

                e3 = ep.tile([P, 4, CHUNK], f16, tag="e3")
                for m in range(4):
                    pt = psB.tile([P, CHUNK], f32, tag="psB")
                    for k in range(4):
                        nc.tensor.matmul(pt[:, :cw], wcct[:, k, m * P:(m + 1) * P],
                                         e2[:, k, :cw], start=(k == 0), stop=(k == 3))
                    nc.vector.tensor_scalar(e3[:, m, :cw], pt[:, :cw],
                                            bias("bcc", m), 0.0, add, amax)

                po = psA.tile([P, CHUNK], f32, tag="psA")
                for k in range(4):
                    nc.tensor.matmul(po[:, :cw], woutt[:, k, :], e3[:, k, :cw],
                                     start=(k == 0), stop=(k == 3))
                half = cc % 2
                if half == 0:
                    ob = outp.tile([2, 2 * CHUNK], f32, tag="ob")
                    ob_f0 = f0
                nc.vector.tensor_scalar_add(
                    ob[:, f0 - ob_f0:f0 - ob_f0 + cw], po[:2, :cw],
                    bias("bout", 0)[:2])
                if half == 1 or cc == len(chunk_list) - 1:
                    nc.sync.dma_start(y[:, ob_f0:f0 + cw],
                                      ob[:, :f0 + cw - ob_f0])

    nc.compile()
    return nc


_cache = {}


def _get_nc():
    if "nc" not in _cache:
        _cache["nc"] = _build()
    return _cache["nc"]


def kernel(brick_vectors, xy, W_xy, b_xy, W_a, b_a, W_b, b_b,
           W_ca, b_ca, W_cb, b_cb, W_cc, b_cc, W_out, b_out):
    # force plain numpy up front (inputs may arrive as jax arrays)
    args = [np.asarray(a) for a in
            (brick_vectors, xy, W_xy, b_xy, W_a, b_a, W_b, b_b,
             W_ca, b_ca, W_cb, b_cb, W_cc, b_cc, W_out, b_out)]
    perms, in_maps = make_in_maps(*args)

    nc = _get_nc()
    res = run_bass_kernel_spmd(nc, in_maps, list(range(NCORES)))

    out = np.empty((B, N, N, 2), np.float32)
    for c in range(NCORES):
        b, perm = perms[c]
        yc = res.results[c]["y"].reshape(2, RLOC, N)       # [2, rl, jj]
        out[b][np.ix_(perm[:RLOC], perm)] = yc.transpose(1, 2, 0)
    return out


# revision 24
# speedup vs baseline: 1.0057x; 1.0057x over previous
"""Trainium2 Bass kernel for nn_BrickVectorEdgeModel (GNN edge MLP).

Computes, for each batch b and node pair (i, j):
    f   = relu(W_b @ relu(W_a @ bv + b_a + W_xy @ xy + b_xy) + b_b)   per node
    e1  = relu(W1 @ f[j] + W2 @ f[i] + b_ca)                          per edge
    e2  = relu(W_cb @ e1 + b_cb)
    e3  = relu(W_cc @ e2 + b_cc)
    out = W_out @ e3 + b_out                                          (2 channels)

Sharding: the (B=4, N=192) x N edge grid has 768 i-rows; each of the 8
cores takes 96 consecutive rows, which always fall inside a single batch
b = core//2.  Host permutes that batch's 192 nodes so the core's own 96
i-rows come first; every core then runs the identical program (SPMD) on
its own node set.  Matmuls run in fp16 (fp32 PSUM accumulate).

The kernel is PE-streaming-bound (~99.6% tensor occupancy), so the focus
is trimming non-stream time:
 - node phase streams 192 cols (true node count) instead of 256-pad;
   vpb streams only the core's own 96 rows.
 - xy contribution enters as a 2-partition matmul (no 128-row zero pad),
   W_out as a 2-column stationary (psum rows 0-1 only).
 - weight DMA is split across three queues (scalar-HWDGE carries the
   f1 critical path, gpsimd-SWDGE the f2/u/v weights, sync the edge
   weights) in consumption order, so the first matmul starts as soon as
   the first ~180KB land instead of after the full 3.4MB.
 - a few zero warmup matmuls run during the initial DMA wait to start
   the tensor-engine DVFS ramp early.
"""

import numpy as np

import concourse.bass as bass
import concourse.mybir as mybir
import concourse.tile as tile
from concourse import bacc
from concourse.bass_utils import run_bass_kernel_spmd

P = 128
H = 512          # hidden width
D = 512          # brick vector dim
B = 4
N = 192          # nodes per batch
NCORES = 8
RLOC = 96        # edge-grid rows per core
EDGES = RLOC * N             # flat edge columns per core (18432)
CHUNK = 512
NCHUNK = EDGES // CHUNK      # 36
VW = RLOC        # vpb columns (own rows only)

WCOLS = 24 * H + 4 * P       # wcat4|wb|w1|w2|wcb|wcc (4*H each) + wout (4*128)

# bias blob layout (fp32): [128 x BCOLS]
_blayout = [("b1", 4), ("bb", 4), ("bca", 4), ("bcb", 4), ("bcc", 4), ("bout", 1)]
BOFF = {}
_c = 0
for _n, _s in _blayout:
    BOFF[_n] = (_c, _s)
    _c += _s
BCOLS = _c


def _to_tiles(w):
    """[K, M] (K = 4*128 contraction) -> [128, 4, M] stationary layout."""
    K, M = w.shape
    return w.reshape(K // P, P, M).transpose(1, 0, 2)


def _pack_weights(W_xy, b_xy, W_a, b_a, W_b, b_b, W_ca, b_ca, W_cb, b_cb,
                  W_cc, b_cc, W_out, b_out):
    blob = np.zeros((P, WCOLS), np.float16)

    def put(idx, arr3):  # arr3: [128, nk, M] -> block idx (4*H cols each)
        blob[:, idx * 4 * H:(idx + 1) * 4 * H] = \
            arr3.reshape(P, -1).astype(np.float16)

    put(0, _to_tiles(W_a.T.astype(np.float32)))       # wcat4 (bv part only)
    put(1, _to_tiles(W_b.T.astype(np.float32)))
    W1, W2 = W_ca[:, :H], W_ca[:, H:]
    put(2, _to_tiles(W1.T.astype(np.float32)))
    put(3, _to_tiles(W2.T.astype(np.float32)))
    put(4, _to_tiles(W_cb.T.astype(np.float32)))
    put(5, _to_tiles(W_cc.T.astype(np.float32)))
    wout = np.zeros((H, P), np.float32)
    wout[:, 0:2] = W_out.T.astype(np.float32)
    blob[:, 24 * H:] = _to_tiles(wout).reshape(P, -1).astype(np.float16)

    bblob = np.zeros((P, BCOLS), np.float32)

    def putb(name, vec):  # [512] -> [128, 4]
        off, sz = BOFF[name]
        bblob[:, off:off + sz] = vec.astype(np.float32).reshape(4, P).T

    putb("b1", np.asarray(b_a) + np.asarray(b_xy))
    putb("bb", b_b)
    putb("bca", b_ca)
    putb("bcb", b_cb)
    putb("bcc", b_cc)
    off, _ = BOFF["bout"]
    bblob[0:2, off] = np.asarray(b_out, np.float32)
    return blob, bblob


def _pack_nodes(W_a, bv_b, perm):
    """Per-core stage1 blob [128, 4, H+N] fp16: [wcat_k | bv^T_k] per k."""
    s = np.zeros((P, 4, H + N), np.float16)
    s[:, :, :H] = _to_tiles(W_a.T.astype(np.float32)).astype(np.float16)
    bvT = bv_b[perm].T.astype(np.float32)          # [512, 192]
    s[:, :, H:] = bvT.reshape(4, P, N).transpose(1, 0, 2).astype(np.float16)
    return s


def _pack_xy(W_xy, xy_b, perm):
    """Per-core 2-partition strip [2, H + N]: W_xy^T | xy^T."""
    s = np.zeros((2, H + N), np.float16)
    s[:, :H] = W_xy.T.astype(np.float16)
    s[:, H:] = xy_b[perm].T.astype(np.float16)
    return s


def make_in_maps(brick_vectors, xy, W_xy, b_xy, W_a, b_a, W_b, b_b,
                 W_ca, b_ca, W_cb, b_cb, W_cc, b_cc, W_out, b_out):
    blob, bblob = _pack_weights(W_xy, b_xy, W_a, b_a, W_b, b_b, W_ca, b_ca,
                                W_cb, b_cb, W_cc, b_cc, W_out, b_out)
    perms, in_maps = [], []
    for c in range(NCORES):
        b, half = c // 2, c % 2
        perm = np.concatenate([np.arange(96) + 96 * half,
                               np.arange(96) + 96 * (1 - half)])
        perms.append((b, perm))
        in_maps.append({
            "wblob": blob,
            "bblob": bblob,
            "nodes": _pack_nodes(np.asarray(W_a),
                                 np.asarray(brick_vectors, np.float32)[b],
                                 perm),
            "xyblob": _pack_xy(np.asarray(W_xy), np.asarray(xy, np.float32)[b],
                               perm),
        })
    return perms, in_maps


def _build():
    f32 = mybir.dt.float32
    Relu = mybir.ActivationFunctionType.Relu
    add = mybir.AluOpType.add
    amax = mybir.AluOpType.max

    f16 = mybir.dt.float16
    nc = bacc.Bacc(None, target_bir_lowering=False)
    wblob = nc.declare_dram_parameter("wblob", [P, WCOLS], f16, isOutput=False)
    bblob = nc.declare_dram_parameter("bblob", [P, BCOLS], f32, isOutput=False)
    nodes = nc.declare_dram_parameter("nodes", [P, 4, H + N], f16,
                                      isOutput=False)
    xyblob = nc.declare_dram_parameter("xyblob", [2, H + N], f16, isOutput=False)
    y = nc.declare_dram_parameter("y", [2, EDGES], f32, isOutput=True)

    with tile.TileContext(nc) as tc:
        with tc.tile_pool(name="wf", bufs=1) as wf, \
             tc.tile_pool(name="stp", bufs=1) as stp, \
             tc.tile_pool(name="wr", bufs=1) as wr, \
             tc.tile_pool(name="ep", bufs=2) as ep, \
             tc.tile_pool(name="outp", bufs=3) as outp, \
             tc.tile_pool(name="psA", bufs=4, space="PSUM") as psA, \
             tc.tile_pool(name="psB", bufs=4, space="PSUM") as psB:

            # warmup operands (zeroed so CoreSim sees defined data)
            warm_s = wf.tile([P, 32], f16, tag="warm_s")
            warm_m = wf.tile([P, CHUNK], f16, tag="warm_m")
            nc.gpsimd.memset(warm_s[:], 0.0)
            nc.gpsimd.memset(warm_m[:], 0.0)

            # stage1 holds [wcat_k | nodes_k] per k-tile: one 5632B-row DMA
            st1 = stp.tile([P, 4, H + N], f16, tag="st1")
            xyt = wf.tile([2, H + N], f16, tag="xyt")
            biast = wf.tile([P, BCOLS], f32, tag="bias")
            wbt = stp.tile([P, 4, H], f16, tag="wb")
            w1t = stp.tile([P, 4, H], f16, tag="w1")
            w2t = stp.tile([P, 4, H], f16, tag="w2")
            wcbt = stp.tile([P, 4, H], f16, tag="wcb")
            wcct = stp.tile([P, 4, H], f16, tag="wcc")
            woutt = stp.tile([P, 4, P], f16, tag="wout")

            # ---- DMA triggers: sync HWDGE (fast, ~190GB/s w/ 4KB rows)
            # carries the critical path in consumption order; scalar HWDGE
            # and gpsimd SWDGE prefetch later-consumed weights in parallel.
            nc.sync.dma_start(st1[:], nodes[:])
            nc.sync.dma_start(wbt[:], wblob[:, 4 * H:8 * H])
            nc.sync.dma_start(wcbt[:], wblob[:, 16 * H:20 * H])
            nc.scalar.dma_start(biast[:], bblob[:])
            nc.scalar.dma_start(xyt[:], xyblob[:])
            nc.scalar.dma_start(w2t[:], wblob[:, 12 * H:16 * H])
            nc.scalar.dma_start(woutt[:], wblob[:, 24 * H:])
            nc.gpsimd.dma_start(w1t[:], wblob[:, 8 * H:12 * H])
            nc.gpsimd.dma_start(wcct[:], wblob[:, 20 * H:24 * H])

            def bias(name, m):
                off, _ = BOFF[name]
                return biast[:, off + m:off + m + 1]

            # ---- PE warmup: start the DVFS ramp during the DMA wait ----
            pw = psA.tile([P, CHUNK], f32, tag="psA")
            for _ in range(8):
                nc.tensor.matmul(pw[:32, :CHUNK], warm_s[:, :32], warm_m[:],
                                 start=True, stop=True)

            # ---- node phase ----
            # f1 (k-outer so matmuls gate on per-k weight slices)
            f1 = wr.tile([P, 4, N], f16, tag="f1")
            pts1 = [psA.tile([P, CHUNK], f32, tag="psA", name=f"pts1_{m}")
                    for m in range(4)]
            for k in range(4):
                for m in range(4):
                    nc.tensor.matmul(pts1[m][:, :N],
                                     st1[:, k, m * P:(m + 1) * P],
                                     st1[:, k, H:], start=(k == 0), stop=False)
            for m in range(4):
                nc.tensor.matmul(pts1[m][:, :N], xyt[0:2, m * P:(m + 1) * P],
                                 xyt[0:2, H:H + N], start=False, stop=True)
            for m in range(4):
                if m % 2 == 0:
                    nc.scalar.activation(f1[:, m, :], pts1[m][:, :N], Relu,
                                         bias=bias("b1", m), scale=1.0)
                else:
                    nc.vector.tensor_scalar(f1[:, m, :], pts1[m][:, :N],
                                            bias("b1", m), 0.0, add, amax)

            # f2
            f2 = wr.tile([P, 4, N], f16, tag="f2")
            pts2 = [psB.tile([P, CHUNK], f32, tag="psB", name=f"pts2_{m}")
                    for m in range(4)]
            for k in range(4):
                for m in range(4):
                    nc.tensor.matmul(pts2[m][:, :N], wbt[:, k, m * P:(m + 1) * P],
                                     f1[:, k, :], start=(k == 0), stop=(k == 3))
            for m in range(4):
                if m % 2 == 0:
                    nc.scalar.activation(f2[:, m, :], pts2[m][:, :N], Relu,
                                         bias=bias("bb", m), scale=1.0)
                else:
                    nc.vector.tensor_scalar(f2[:, m, :], pts2[m][:, :N],
                                            bias("bb", m), 0.0, add, amax)

            # u (all 192 j-nodes) and vpb = W2@f2 + b_ca (own 96 i-rows only)
            u = wr.tile([P, 4, N], f32, tag="u")
            vpb = wr.tile([P, 4, VW], f32, tag="vpb")
            for mm in range(4):
                pu = psA.tile([P, CHUNK], f32, tag="psA")
                for k in range(4):
                    nc.tensor.matmul(pu[:, :N], w1t[:, k, mm * P:(mm + 1) * P],
                                     f2[:, k, :], start=(k == 0), stop=(k == 3))
                if mm < 2:
                    nc.scalar.copy(u[:, mm, :], pu[:, :N])
                else:
                    nc.vector.tensor_copy(u[:, mm, :], pu[:, :N])
                pv = psB.tile([P, CHUNK], f32, tag="psB")
                for k in range(4):
                    nc.tensor.matmul(pv[:, :VW],
                                     w2t[:, k, mm * P:(mm + 1) * P],
                                     f2[:, k, :VW], start=(k == 0), stop=(k == 3))
                nc.vector.tensor_scalar_add(vpb[:, mm, :], pv[:, :VW],
                                            bias("bca", mm))

            # ---- edge phase: 512-wide chunks; last chunk split in two to
            #      shorten the serial relu->out tail after the final matmul ----
            chunk_list = [(cc * CHUNK, CHUNK) for cc in range(NCHUNK - 1)]
            chunk_list += [((NCHUNK - 1) * CHUNK, CHUNK // 2),
                           ((NCHUNK - 1) * CHUNK + CHUNK // 2, CHUNK // 2)]
            for cc, (f0, cw) in enumerate(chunk_list):
                e1 = ep.tile([P, 4, CHUNK], f16, tag="e1")
                r_lo = f0 // N
                r_hi = (f0 + cw - 1) // N
                for kt in range(4):
                    for rl in range(r_lo, r_hi + 1):
                        cs = max(f0, rl * N)
                        ce = min(f0 + cw, (rl + 1) * N)
                        if cc == 0 and kt >= 2:
                            nc.vector.tensor_scalar(
                                e1[:, kt, cs - f0:ce - f0],
                                u[:, kt, cs - rl * N:ce - rl * N],
                                vpb[:, kt, rl:rl + 1], 0.0, add, amax)
                        else:
                            nc.scalar.activation(
                                e1[:, kt, cs - f0:ce - f0],
                                u[:, kt, cs - rl * N:ce - rl * N],
                                Relu, bias=vpb[:, kt, rl:rl + 1], scale=1.0)

                e2 = ep.tile([P, 4, CHUNK], f16, tag="e2")
                for m in range(4):
                    pt = psA.tile([P, CHUNK], f32, tag="psA")
                    for k in range(4):
                        nc.tensor.matmul(pt[:, :cw], wcbt[:, k, m * P:(m + 1) * P],
                                         e1[:, k, :cw], start=(k == 0), stop=(k == 3))
                    nc.vector.tensor_scalar(e2[:, m, :cw], pt[:, :cw],
                                            bias("bcb", m), 0.0, add, amax)

                e3 = ep.tile([P, 4, CHUNK], f16, tag="e3")
                for m in range(4):
                    pt = psB.tile([P, CHUNK], f32, tag="psB")
                    for k in range(4):
                        nc.tensor.matmul(pt[:, :cw], wcct[:, k, m * P:(m + 1) * P],
                                         e2[:, k, :cw], start=(k == 0), stop=(k == 3))
                    nc.vector.tensor_scalar(e3[:, m, :cw], pt[:, :cw],
                                            bias("bcc", m), 0.0, add, amax)

                po = psA.tile([P, CHUNK], f32, tag="psA")
                for k in range(4):
                    nc.tensor.matmul(po[:, :cw], woutt[:, k, :], e3[:, k, :cw],
                                     start=(k == 0), stop=(k == 3))
                half = cc % 2
                if half == 0:
                    ob = outp.tile([2, 2 * CHUNK], f32, tag="ob")
                    ob_f0 = f0
                nc.vector.tensor_scalar_add(
                    ob[:, f0 - ob_f0:f0 - ob_f0 + cw], po[:2, :cw],
                    bias("bout", 0)[:2])
                if half == 1 or cc == len(chunk_list) - 1:
                    nc.sync.dma_start(y[:, ob_f0:f0 + cw],
                                      ob[:, :f0 + cw - ob_f0])

    nc.compile()
    return nc


_cache = {}


def _get_nc():
    if "nc" not in _cache:
        _cache["nc"] = _build()
    return _cache["nc"]


def kernel(brick_vectors, xy, W_xy, b_xy, W_a, b_a, W_b, b_b,
           W_ca, b_ca, W_cb, b_cb, W_cc, b_cc, W_out, b_out):
    # force plain numpy up front (inputs may arrive as jax arrays)
    args = [np.asarray(a) for a in
            (brick_vectors, xy, W_xy, b_xy, W_a, b_a, W_b, b_b,
             W_ca, b_ca, W_cb, b_cb, W_cc, b_cc, W_out, b_out)]
    perms, in_maps = make_in_maps(*args)

    nc = _get_nc()
    res = run_bass_kernel_spmd(nc, in_maps, list(range(NCORES)))

    out = np.empty((B, N, N, 2), np.float32)
    for c in range(NCORES):
        b, perm = perms[c]
        yc = res.results[c]["y"].reshape(2, RLOC, N)       # [2, rl, jj]
        out[b][np.ix_(perm[:RLOC], perm)] = yc.transpose(1, 2, 0)
    return out
